# revision 5
# baseline (speedup 1.0000x reference)
"""Walsh-Hadamard transform (4096-point, orthonormal) on trn2, 8 cores.

y[r] = (H_4096 @ x[r]) / 64  for each of 16384 rows.

Scheme: H_4096 = H_8 (x) H_4 (x) H_128 over n = i*512 + v*128 + u
(i in 8, v in 4, u in 128). Rows are processed in groups of 16. An SBUF
tile holds a 16-row group as [128 partitions = (rr*8 + i), 512 free =
(v,u)]; each partition row is one contiguous 2 KiB chunk of DRAM (512
f32), which keeps DMA descriptors near line-rate efficiency.

Compute is in bf16 (the f32->bf16 cast happens inline in the SDMA
engines during the load, via SWDGE cast-DMA; all Hadamard factors are
exactly representable: +-1 and +-2^-6), accumulation in fp32 PSUM. Two
matmul stages per group:
  mm1 (x4, one per v): out1_v = Xslice_v.T @ BD   (BD = I_16 (x) H_8)
      -> [u, (rr,a)] in PSUM; the data is the stationary operand so the
      matmul also performs the layout corner-turn.
  mm2 (x4, accumulating, N=512): ps2 += t1_v.T @ M_v with
      M_v[u, v'*128+u'] = H4[v',v] * H128[u,u'] / 64
      -> [(rr,a), (v',u')] which is exactly the natural row-major output
      layout, so the store is also plain 2 KiB-chunk DMAs.

Work is sharded row-wise: core c processes rows [c*2048, (c+1)*2048).
"""

import numpy as np

N_ROWS = 16384
DIM = 4096
N_CORES = 8
R_PER_CORE = N_ROWS // N_CORES  # 2048

G = 4  # 16-row groups per DMA chunk -> 64 rows = 1 MiB per direction

_PROG_CACHE = {}


def _hadamard(n: int) -> np.ndarray:
    H = np.array([[1.0]], dtype=np.float64)
    while H.shape[0] < n:
        H = np.block([[H, H], [H, -H]])
    return H


def _build_program():
    import concourse.mybir as mybir
    from concourse import bacc
    from concourse.tile import TileContext

    f32 = mybir.dt.float32
    bf16 = mybir.dt.bfloat16
    nc = bacc.Bacc("TRN2")

    x = nc.declare_dram_parameter("x", [R_PER_CORE, DIM], f32, isOutput=False)
    y = nc.declare_dram_parameter("y", [R_PER_CORE, DIM], f32, isOutput=True)

    BD = np.kron(np.eye(16), _hadamard(8)).astype(np.float32)  # [(rr,i),(rr,a)]
    Hs = _hadamard(128) / 64.0  # [u, u']
    H4 = _hadamard(4)  # [v', v]
    # M_v[u, v'*128+u'] = H4[v',v] * Hs[u,u']
    Ms = [
        np.concatenate([H4[vp, v] * Hs for vp in range(4)], axis=1).astype(
            np.float32
        )
        for v in range(4)
    ]

    bd_d = nc.inline_tensor(BD, "bd_const")
    m_d = [nc.inline_tensor(Ms[v], f"m{v}_const") for v in range(4)]

    n_chunks = R_PER_CORE // (16 * G)  # 32

    xv = x[:].rearrange("(cb g rr) (i jj) -> cb (rr i) g jj", g=G, rr=16, i=8, jj=512)
    yv = y[:].rearrange("(cb g rr) (a jj) -> cb (rr a) g jj", g=G, rr=16, a=8, jj=512)

    with TileContext(nc) as tc:
        with (
            tc.tile_pool(name="consts", bufs=1) as cpool,
            tc.tile_pool(name="inbf", bufs=4) as bfpool,
            tc.tile_pool(name="outp", bufs=3) as outpool,
            tc.tile_pool(name="mid", bufs=6) as midpool,
            tc.tile_pool(name="ps1", bufs=4, space="PSUM") as ps1pool,
            tc.tile_pool(name="ps2", bufs=4, space="PSUM") as ps2pool,
        ):
            bd_f = cpool.tile([128, 128], f32)
            nc.sync.dma_start(out=bd_f[:], in_=bd_d[:])
            bd_sb = cpool.tile([128, 128], bf16)
            nc.vector.tensor_copy(out=bd_sb[:], in_=bd_f[:])
            m_sb = []
            for v in range(4):
                m_f = cpool.tile([128, 512], f32, tag=f"mf{v}")
                nc.sync.dma_start(out=m_f[:], in_=m_d[v][:])
                m_b = cpool.tile([128, 512], bf16, tag=f"mb{v}")
                nc.vector.tensor_copy(out=m_b[:], in_=m_f[:])
                m_sb.append(m_b)

            for cb in range(n_chunks):
                # SWDGE cast-during-DMA: reads f32 from HBM, lands bf16 in
                # SBUF (the cast runs inline in the SDMA engines).
                in_bf = bfpool.tile([128, G, 512], bf16)
                nc.gpsimd.dma_start(out=in_bf[:], in_=xv[cb])
                out_tile = outpool.tile([128, G, 512], f32)
                for g in range(G):
                    ps1 = ps1pool.tile([128, 512], f32)
                    for v in range(4):
                        nc.tensor.matmul(
                            ps1[:, v * 128 : (v + 1) * 128],
                            in_bf[:, g, v * 128 : (v + 1) * 128],
                            bd_sb[:],
                            start=True,
                            stop=True,
                        )
                    t1 = midpool.tile([128, 512], bf16)
                    nc.scalar.copy(t1[:], ps1[:])
                    ps2 = ps2pool.tile([128, 512], f32)
                    for v in range(4):
                        nc.tensor.matmul(
                            ps2[:],
                            t1[:, v * 128 : (v + 1) * 128],
                            m_sb[v][:],
                            start=(v == 0),
                            stop=(v == 3),
                        )
                    nc.vector.tensor_copy(out=out_tile[:, g], in_=ps2[:])
                nc.sync.dma_start(out=yv[cb], in_=out_tile[:])

    nc.compile()
    return nc


def _get_program():
    if "nc" not in _PROG_CACHE:
        _PROG_CACHE["nc"] = _build_program()
    return _PROG_CACHE["nc"]


def kernel(x, _trace=False, _trace_kwargs=None):
    from concourse.bass_utils import run_bass_kernel_spmd

    x = np.ascontiguousarray(np.asarray(x, dtype=np.float32))
    assert x.shape == (N_ROWS, DIM), x.shape

    nc = _get_program()
    core_ids = list(range(N_CORES))
    in_maps = [
        {"x": x[c * R_PER_CORE : (c + 1) * R_PER_CORE]} for c in core_ids
    ]
    res = run_bass_kernel_spmd(
        nc, in_maps, core_ids, trace=_trace, **(_trace_kwargs or {})
    )
    out = np.concatenate([r["y"] for r in res.results], axis=0)
    if _trace:
        return out, res
    return out


# revision 6
# speedup vs baseline: 1.0117x; 1.0117x over previous
"""Walsh-Hadamard transform (4096-point, orthonormal) on trn2, 8 cores.

y[r] = (H_4096 @ x[r]) / 64  for each of 16384 rows.

Scheme: H_4096 = H_16 (x) H_2 (x) H_128 over n = i*256 + v*128 + u
(i in 16, v in 2, u in 128). Rows are processed in groups of 8. An SBUF
tile holds an 8-row group as [128 partitions = (rr*16 + i), 256 free =
(v,u)]; each partition row is one contiguous 1 KiB chunk of DRAM (256
f32), which keeps DMA descriptors at full HBM-side efficiency.

Compute is in bf16 (the f32->bf16 cast happens inline in the SDMA
engines during the load, via SWDGE cast-DMA; all Hadamard factors are
exactly representable: +-1 and +-2^-6), accumulation in fp32 PSUM. Two
matmul stages per group:
  mm1 (x2, one per v): out1_v = Xslice_v.T @ BD   (BD = I_8 (x) H_16)
      -> [u, (rr,a)] in PSUM; the data is the stationary operand so the
      matmul also performs the layout corner-turn.
  mm2 (x2, accumulating, N=256): ps2 += t1_v.T @ M_v with
      M_0 = [Hs | Hs], M_1 = [Hs | -Hs], Hs = H_128/64
      -> [(rr,a), (v',u')] which is exactly the natural row-major output
      layout, so the store is also plain 1 KiB-chunk DMAs.

The first two chunks are loaded as f32 via HWDGE (which becomes ready
~3 us before the SWDGE path) and converted on DVE/ACT, so the DMA
engines start streaming as early as possible.

Work is sharded row-wise: core c processes rows [c*2048, (c+1)*2048).
"""

import numpy as np

N_ROWS = 16384
DIM = 4096
N_CORES = 8
R_PER_CORE = N_ROWS // N_CORES  # 2048

G = 8  # 8-row groups per DMA chunk -> 64 rows = 1 MiB per direction
SB = 2  # groups per PSUM bank (2 * 256 fp32 = 512 = one bank)
PREFETCH = 2  # leading chunks loaded via HWDGE f32 + on-chip convert

_PROG_CACHE = {}


def _hadamard(n: int) -> np.ndarray:
    H = np.array([[1.0]], dtype=np.float64)
    while H.shape[0] < n:
        H = np.block([[H, H], [H, -H]])
    return H


def _build_program():
    import concourse.mybir as mybir
    from concourse import bacc
    from concourse.tile import TileContext

    f32 = mybir.dt.float32
    bf16 = mybir.dt.bfloat16
    nc = bacc.Bacc("TRN2")

    x = nc.declare_dram_parameter("x", [R_PER_CORE, DIM], f32, isOutput=False)
    y = nc.declare_dram_parameter("y", [R_PER_CORE, DIM], f32, isOutput=True)

    BD = np.kron(np.eye(8), _hadamard(16)).astype(np.float32)  # [(rr,i),(rr,a)]
    Hs = _hadamard(128) / 64.0  # [u, u']
    M0 = np.concatenate([Hs, Hs], axis=1).astype(np.float32)  # [u, (v',u')]
    M1 = np.concatenate([Hs, -Hs], axis=1).astype(np.float32)

    bd_d = nc.inline_tensor(BD.astype(np.float32), "bd_const")
    m0_d = nc.inline_tensor(M0, "m0_const")
    m1_d = nc.inline_tensor(M1, "m1_const")

    n_chunks = R_PER_CORE // (8 * G)  # 32

    xv = x[:].rearrange("(cb g rr) (i jj) -> cb (rr i) g jj", g=G, rr=8, i=16, jj=256)
    yv = y[:].rearrange("(cb g rr) (a jj) -> cb (rr a) g jj", g=G, rr=8, a=16, jj=256)

    with TileContext(nc) as tc:
        with (
            tc.tile_pool(name="consts", bufs=1) as cpool,
            tc.tile_pool(name="pf", bufs=PREFETCH) as pfpool,
            tc.tile_pool(name="inbf", bufs=4) as bfpool,
            tc.tile_pool(name="outp", bufs=3) as outpool,
            tc.tile_pool(name="mid", bufs=8) as midpool,
            tc.tile_pool(name="ps1", bufs=4, space="PSUM") as ps1pool,
            tc.tile_pool(name="ps2", bufs=4, space="PSUM") as ps2pool,
        ):
            # HWDGE prefetch of the leading chunks (f32), queued first so the
            # DMA engines have work before the SWDGE path warms up.
            pf_tiles = []
            for cb in range(PREFETCH):
                in_f = pfpool.tile([128, G, 256], f32)
                nc.sync.dma_start(out=in_f[:], in_=xv[cb])
                pf_tiles.append(in_f)

            bd_f = cpool.tile([128, 128], f32)
            m0_f = cpool.tile([128, 256], f32)
            m1_f = cpool.tile([128, 256], f32)
            nc.sync.dma_start(out=bd_f[:], in_=bd_d[:])
            nc.sync.dma_start(out=m0_f[:], in_=m0_d[:])
            nc.sync.dma_start(out=m1_f[:], in_=m1_d[:])
            bd_sb = cpool.tile([128, 128], bf16)
            m0_sb = cpool.tile([128, 256], bf16)
            m1_sb = cpool.tile([128, 256], bf16)
            nc.vector.tensor_copy(out=bd_sb[:], in_=bd_f[:])
            nc.vector.tensor_copy(out=m0_sb[:], in_=m0_f[:])
            nc.vector.tensor_copy(out=m1_sb[:], in_=m1_f[:])

            for cb in range(n_chunks):
                in_bf = bfpool.tile([128, G, 256], bf16)
                if cb < PREFETCH:
                    # Convert the prefetched f32 chunk on DVE/ACT (idle early).
                    if cb % 2 == 0:
                        nc.vector.tensor_copy(out=in_bf[:], in_=pf_tiles[cb][:])
                    else:
                        nc.scalar.copy(in_bf[:], pf_tiles[cb][:])
                else:
                    # SWDGE cast-during-DMA: reads f32 from HBM, lands bf16
                    # in SBUF (the cast runs inline in the SDMA engines).
                    nc.gpsimd.dma_start(out=in_bf[:], in_=xv[cb])
                out_tile = outpool.tile([128, G, 256], f32)
                for s in range(G // SB):
                    ps1 = ps1pool.tile([128, SB * 256], f32)
                    for g2 in range(SB):
                        g = s * SB + g2
                        for v in range(2):
                            nc.tensor.matmul(
                                ps1[:, g2 * 256 + v * 128 : g2 * 256 + (v + 1) * 128],
                                in_bf[:, g, v * 128 : (v + 1) * 128],
                                bd_sb[:],
                                start=True,
                                stop=True,
                            )
                    t1 = midpool.tile([128, SB * 256], bf16)
                    nc.scalar.copy(t1[:], ps1[:])
                    ps2 = ps2pool.tile([128, SB * 256], f32)
                    for g2 in range(SB):
                        nc.tensor.matmul(
                            ps2[:, g2 * 256 : (g2 + 1) * 256],
                            t1[:, g2 * 256 : g2 * 256 + 128],
                            m0_sb[:],
                            start=True,
                            stop=False,
                        )
                        nc.tensor.matmul(
                            ps2[:, g2 * 256 : (g2 + 1) * 256],
                            t1[:, g2 * 256 + 128 : g2 * 256 + 256],
                            m1_sb[:],
                            start=False,
                            stop=True,
                        )
                    nc.vector.tensor_copy(
                        out=out_tile[:, s * SB : (s + 1) * SB].rearrange(
                            "p g c -> p (g c)"
                        ),
                        in_=ps2[:],
                    )
                nc.sync.dma_start(out=yv[cb], in_=out_tile[:])

    nc.compile()
    return nc


def _get_program():
    if "nc" not in _PROG_CACHE:
        _PROG_CACHE["nc"] = _build_program()
    return _PROG_CACHE["nc"]


def kernel(x, _trace=False, _trace_kwargs=None):
    from concourse.bass_utils import run_bass_kernel_spmd

    x = np.ascontiguousarray(np.asarray(x, dtype=np.float32))
    assert x.shape == (N_ROWS, DIM), x.shape

    nc = _get_program()
    core_ids = list(range(N_CORES))
    in_maps = [
        {"x": x[c * R_PER_CORE : (c + 1) * R_PER_CORE]} for c in core_ids
    ]
    res = run_bass_kernel_spmd(
        nc, in_maps, core_ids, trace=_trace, **(_trace_kwargs or {})
    )
    out = np.concatenate([r["y"] for r in res.results], axis=0)
    if _trace:
        return out, res
    return out


# revision 7
# speedup vs baseline: 1.0580x; 1.0458x over previous
"""Walsh-Hadamard transform (4096-point, orthonormal) on trn2, 8 cores.

y[r] = (H_4096 @ x[r]) / 64  for each of 16384 rows.

Scheme: H_4096 = H_16 (x) H_2 (x) H_128 over n = i*256 + v*128 + u
(i in 16, v in 2, u in 128). Rows are processed in groups of 8. An SBUF
tile holds an 8-row group as [128 partitions = (rr*16 + i), 256 free =
(v,u)]; each partition row is one contiguous 1 KiB chunk of DRAM (256
f32), which keeps DMA descriptors at full HBM-side efficiency.

Compute is in bf16 (the f32->bf16 cast happens inline in the SDMA
engines during the load, via SWDGE cast-DMA; all Hadamard factors are
exactly representable: +-1 and +-2^-6), accumulation in fp32 PSUM. Two
matmul stages per group:
  mm1 (x2, one per v): out1_v = Xslice_v.T @ BD   (BD = I_8 (x) H_16)
      -> [u, (rr,a)] in PSUM; the data is the stationary operand so the
      matmul also performs the layout corner-turn.
  mm2 (x2, accumulating, N=256): ps2 += t1_v.T @ M_v with
      M_0 = [Hs | Hs], M_1 = [Hs | -Hs], Hs = H_128/64
      -> [(rr,a), (v',u')] which is exactly the natural row-major output
      layout, so the store is also plain 1 KiB-chunk DMAs.

Work is sharded row-wise: core c processes rows [c*2048, (c+1)*2048).
"""

import numpy as np

N_ROWS = 16384
DIM = 4096
N_CORES = 8
R_PER_CORE = N_ROWS // N_CORES  # 2048

G = 8  # 8-row groups per DMA chunk -> 64 rows = 1 MiB per direction
SB = 2  # groups per PSUM bank (2 * 256 fp32 = 512 = one bank)

_PROG_CACHE = {}


def _hadamard(n: int) -> np.ndarray:
    H = np.array([[1.0]], dtype=np.float64)
    while H.shape[0] < n:
        H = np.block([[H, H], [H, -H]])
    return H


def _build_program():
    import concourse.mybir as mybir
    from concourse import bacc
    from concourse.tile import TileContext

    f32 = mybir.dt.float32
    bf16 = mybir.dt.bfloat16
    nc = bacc.Bacc("TRN2")

    x = nc.declare_dram_parameter("x", [R_PER_CORE, DIM], f32, isOutput=False)
    y = nc.declare_dram_parameter("y", [R_PER_CORE, DIM], f32, isOutput=True)

    BD = np.kron(np.eye(8), _hadamard(16)).astype(np.float32)  # [(rr,i),(rr,a)]
    Hs = _hadamard(128) / 64.0  # [u, u']
    M0 = np.concatenate([Hs, Hs], axis=1).astype(np.float32)  # [u, (v',u')]
    M1 = np.concatenate([Hs, -Hs], axis=1).astype(np.float32)

    bd_d = nc.inline_tensor(BD.astype(np.float32), "bd_const")
    m0_d = nc.inline_tensor(M0, "m0_const")
    m1_d = nc.inline_tensor(M1, "m1_const")

    n_chunks = R_PER_CORE // (8 * G)  # 32

    xv = x[:].rearrange("(cb g rr) (i jj) -> cb (rr i) g jj", g=G, rr=8, i=16, jj=256)
    yv = y[:].rearrange("(cb g rr) (a jj) -> cb (rr a) g jj", g=G, rr=8, a=16, jj=256)

    with TileContext(nc) as tc:
        with (
            tc.tile_pool(name="consts", bufs=1) as cpool,
            tc.tile_pool(name="inbf", bufs=4) as bfpool,
            tc.tile_pool(name="outp", bufs=3) as outpool,
            tc.tile_pool(name="mid", bufs=6) as midpool,
            tc.tile_pool(name="ps1", bufs=3, space="PSUM") as ps1pool,
            tc.tile_pool(name="ps2", bufs=3, space="PSUM") as ps2pool,
        ):
            bd_f = cpool.tile([128, 128], f32)
            m0_f = cpool.tile([128, 256], f32)
            m1_f = cpool.tile([128, 256], f32)
            nc.sync.dma_start(out=bd_f[:], in_=bd_d[:])
            nc.sync.dma_start(out=m0_f[:], in_=m0_d[:])
            nc.sync.dma_start(out=m1_f[:], in_=m1_d[:])
            bd_sb = cpool.tile([128, 128], bf16)
            m0_sb = cpool.tile([128, 256], bf16)
            m1_sb = cpool.tile([128, 256], bf16)
            nc.vector.tensor_copy(out=bd_sb[:], in_=bd_f[:])
            nc.vector.tensor_copy(out=m0_sb[:], in_=m0_f[:])
            nc.vector.tensor_copy(out=m1_sb[:], in_=m1_f[:])

            for cb in range(n_chunks):
                # SWDGE cast-during-DMA: reads f32 from HBM, lands bf16 in
                # SBUF (the cast runs inline in the SDMA engines).
                in_bf = bfpool.tile([128, G, 256], bf16)
                nc.gpsimd.dma_start(out=in_bf[:], in_=xv[cb])
                out_tile = outpool.tile([128, G, 256], f32)
                for s in range(G // SB):
                    ps1 = ps1pool.tile([128, SB * 256], f32)
                    for g2 in range(SB):
                        g = s * SB + g2
                        for v in range(2):
                            nc.tensor.matmul(
                                ps1[:, g2 * 256 + v * 128 : g2 * 256 + (v + 1) * 128],
                                in_bf[:, g, v * 128 : (v + 1) * 128],
                                bd_sb[:],
                                start=True,
                                stop=True,
                            )
                    t1 = midpool.tile([128, SB * 256], bf16)
                    nc.scalar.copy(t1[:], ps1[:])
                    ps2 = ps2pool.tile([128, SB * 256], f32)
                    for g2 in range(SB):
                        nc.tensor.matmul(
                            ps2[:, g2 * 256 : (g2 + 1) * 256],
                            t1[:, g2 * 256 : g2 * 256 + 128],
                            m0_sb[:],
                            start=True,
                            stop=False,
                        )
                        nc.tensor.matmul(
                            ps2[:, g2 * 256 : (g2 + 1) * 256],
                            t1[:, g2 * 256 + 128 : g2 * 256 + 256],
                            m1_sb[:],
                            start=False,
                            stop=True,
                        )
                    nc.vector.tensor_copy(
                        out=out_tile[:, s * SB : (s + 1) * SB].rearrange(
                            "p g c -> p (g c)"
                        ),
                        in_=ps2[:],
                    )
                nc.sync.dma_start(out=yv[cb], in_=out_tile[:])

    nc.compile()
    return nc


def _get_program():
    if "nc" not in _PROG_CACHE:
        _PROG_CACHE["nc"] = _build_program()
    return _PROG_CACHE["nc"]


def kernel(x, _trace=False, _trace_kwargs=None):
    from concourse.bass_utils import run_bass_kernel_spmd

    x = np.ascontiguousarray(np.asarray(x, dtype=np.float32))
    assert x.shape == (N_ROWS, DIM), x.shape

    nc = _get_program()
    core_ids = list(range(N_CORES))
    in_maps = [
        {"x": x[c * R_PER_CORE : (c + 1) * R_PER_CORE]} for c in core_ids
    ]
    res = run_bass_kernel_spmd(
        nc, in_maps, core_ids, trace=_trace, **(_trace_kwargs or {})
    )
    out = np.concatenate([r["y"] for r in res.results], axis=0)
    if _trace:
        return out, res
    return out
